# revision 1
# baseline (speedup 1.0000x reference)
"""TRN2 Bass kernel: 3-layer GIN (sum-agg) + MLP + BatchNorm + graph sum-pooling + linear.

Full inputs in, full output out. Internally: 8-way data parallel over nodes
(12500 contiguous nodes/core), SPMD NEFF via run_bass_kernel_spmd.

Per layer l on each core (feature-major f32 working set [64, 12544]):
  l==0: x = (1+eps0)*emb[nid]+agg factors through the 1600-wide vocabulary:
        x^T_blk = sum_vc emb16_chunk^T @ (counts + (1+eps)*onehot(nid)) — no gathers.
  l>0 : agg^T_blk accumulates in PSUM over indirect-DMA row gathers (128 rows/call,
        fp16 h table, two 23-bit windows via element_offset) matmul'ed against
        one-hot dst matrices built on DVE.
  Then 3x(Linear+ReLU) on PE/ACT, BN stats cross-core via AllReduce, per-graph
  pooling via one-hot matmul, h writeback (PE transpose -> fp16 rows) + AllGather.
Final: pooled^T_l @ W_out slices accumulate -> [128,100], indirect-scatter into
[513,100] by graph window, AllReduce over cores, + b_out.
"""
import math
import sys
import types

import numpy as np

HID = 64
P = 128
NCORES = 8
VOCAB = 3100
ID_OFFSET = 1500
NUM_CLASSES = 100
BN_EPS = 1e-5

CFG = dict(
    n_nodes=100_000,
    n_graphs=512,
    win_rows=65_536,   # fp16 row reach of 23-bit dynamic offset
    blk=512,           # dst nodes per PSUM block
    oh_batch=4,        # one-hot tiles built per DVE op
)

_PROFILE = False
_LAST_EXEC_NS = None


def _install_profile_hook():
    try:
        import antenv
        from trn_agent_boot.trn_boot import _ntff_profile_via_ctypes
    except Exception:
        return False
    if "antenv.axon_hooks" in sys.modules:
        return True
    hooks = types.ModuleType("antenv.axon_hooks")
    hooks._hook = _ntff_profile_via_ctypes("/opt/axon/libaxon_pjrt.so")
    hooks.set_axon_ntff_profile_hook = lambda h: setattr(hooks, "_hook", h)
    hooks.get_axon_ntff_profile_hook = lambda: hooks._hook
    sys.modules["antenv.axon_hooks"] = hooks
    antenv.axon_hooks = hooks
    return True


def _derived(cfg):
    n = cfg["n_nodes"]
    loc = n // NCORES
    lpad = ((loc + P - 1) // P) * P
    nblk = (lpad + cfg["blk"] - 1) // cfg["blk"]
    ntn = lpad // P
    vpad = ((VOCAB - ID_OFFSET + P - 1) // P) * P   # 1664
    nvc = vpad // P
    return loc, lpad, nblk, ntn, vpad, nvc


def _prep(cfg, node_ids, edge_src, edge_dst, graph_ids, Ws, bs, bn_gamma,
          bn_beta, eps, W_out, b_out, emb):
    """Index-only host preprocessing -> per-core input dicts + compile caps."""
    loc, lpad, nblk, ntn, vpad, nvc = _derived(cfg)
    n, wr, blk = cfg["n_nodes"], cfg["win_rows"], cfg["blk"]
    node_ids = np.asarray(node_ids, np.int64)
    edge_src = np.asarray(edge_src, np.int64)
    edge_dst = np.asarray(edge_dst, np.int64)
    graph_ids = np.asarray(graph_ids, np.int64)

    per_core = []
    # pass 1: group edges, find caps
    grp = []   # per core: dict (b, w) -> (src_idx_rel, dst_in_blk)
    for c in range(NCORES):
        base = c * loc
        m = (edge_dst >= base) & (edge_dst < base + loc)
        src = edge_src[m]
        dl = edge_dst[m] - base
        b = dl // blk
        w = (src >= wr).astype(np.int64)
        g = {}
        order = np.lexsort((dl, w, b))
        src, dl, b, w = src[order], dl[order], b[order], w[order]
        key = b * 2 + w
        cuts = np.searchsorted(key, np.arange(nblk * 2 + 1))
        for bb in range(nblk):
            for ww in range(2):
                k = bb * 2 + ww
                s, e = cuts[k], cuts[k + 1]
                g[(bb, ww)] = (src[s:e] - ww * wr, dl[s:e] - bb * blk)
        grp.append(g)

    cap = [0, 0]
    for c in range(NCORES):
        for (bb, ww), (s, _) in grp[c].items():
            cap[ww] = max(cap[ww], len(s))
    tiles_w = [max(1, (cap[w] + P - 1) // P) for w in range(2)]
    tot_t = tiles_w[0] + tiles_w[1]
    ncalls = nblk * tot_t

    # counts matrix + slot arrays per core
    for c in range(NCORES):
        base = c * loc
        eidx = np.zeros((P, ncalls), np.int32)
        edst = np.full((P, ncalls), -1.0, np.float32)
        for bb in range(nblk):
            for ww in range(2):
                s, d = grp[c][(bb, ww)]
                cal0 = bb * tot_t + (0 if ww == 0 else tiles_w[0])
                nt_ = tiles_w[ww]
                si = np.zeros(nt_ * P, np.int32)
                di = np.full(nt_ * P, -1.0, np.float32)
                si[:len(s)] = s
                di[:len(s)] = d.astype(np.float32)
                eidx[:, cal0:cal0 + nt_] = si.reshape(nt_, P).T
                edst[:, cal0:cal0 + nt_] = di.reshape(nt_, P).T

        # vocab count matrix cntT [vpad, lpad] fp16
        m = (edge_dst >= base) & (edge_dst < base + loc)
        src = edge_src[m]
        dl = edge_dst[m] - base
        v = node_ids[src]          # 0..VOCAB-ID_OFFSET-1 range by construction
        cnt = np.bincount(v * lpad + dl, minlength=vpad * lpad)
        cntT = cnt.reshape(vpad, lpad).astype(np.float16)

        nidrep = np.full((P, lpad), -1.0, np.float16)
        nidrep[:, :loc] = np.tile(node_ids[base:base + loc].astype(np.float16),
                                  (P, 1))
        gl = np.full((P, ntn), -1.0, np.float32)
        g_base = int(graph_ids[base])
        g_span = int(graph_ids[base + loc - 1]) - g_base
        assert g_span < P, f"graph window {g_span} >= {P}"
        glv = (graph_ids[base:base + loc] - g_base).astype(np.float32)
        gl_full = np.full(lpad, -1.0, np.float32)
        gl_full[:loc] = glv
        gl[:, :] = gl_full.reshape(ntn, P).T
        growidx = np.minimum(g_base + np.arange(P), cfg["n_graphs"]).astype(
            np.int32)[:, None]

        wpack = np.zeros((HID, 9 * HID), np.float32)
        for l in range(3):
            for mm in range(3):
                wpack[:, (3 * l + mm) * HID:(3 * l + mm + 1) * HID] = Ws[l, mm]
        woutp = np.zeros((HID, 3 * NUM_CLASSES), np.float32)
        for l in range(3):
            woutp[:, l * NUM_CLASSES:(l + 1) * NUM_CLASSES] = \
                W_out[l * HID:(l + 1) * HID]

        per_core.append(dict(
            emb=np.asarray(emb, np.float32),
            cntT=cntT,
            nidrep=nidrep,
            eidx=eidx,
            edst=edst,
            gl=gl,
            growidx=growidx,
            iotaV=(np.arange(P, dtype=np.float32)[:, None]
                   + P * np.arange(nvc, dtype=np.float32)[None, :]),
            iota512=np.tile(np.arange(blk, dtype=np.float32)[None, :], (P, 1)),
            iota128=np.tile(np.arange(P, dtype=np.float32)[None, :], (P, 1)),
            idn64=np.eye(HID, dtype=np.float32),
            wpack=wpack,
            bsT=np.asarray(bs, np.float32).reshape(9, HID).T.copy(),
            gammaT=np.asarray(bn_gamma, np.float32).T.copy(),
            betaT=np.asarray(bn_beta, np.float32).T.copy(),
            eps_rep=np.tile(np.asarray(eps, np.float32)[None, :], (P, 1)),
            woutp=woutp,
            boutr=np.tile(np.asarray(b_out, np.float32)[None, :], (P, 1)),
        ))
    return per_core, tiles_w, ncalls


def _build(cfg, tiles_w, ncalls):
    import concourse.bacc as bacc
    import concourse.bass as bass
    import concourse.mybir as mybir
    import concourse.tile as tile

    loc, lpad, nblk, ntn, vpad, nvc = _derived(cfg)
    n, g, wr, blk = (cfg["n_nodes"], cfg["n_graphs"], cfg["win_rows"],
                     cfg["blk"])
    OHB = cfg["oh_batch"]
    f32, f16, i32 = mybir.dt.float32, mybir.dt.float16, mybir.dt.int32
    AL, AF = mybir.AluOpType, mybir.ActivationFunctionType
    tot_t = tiles_w[0] + tiles_w[1]
    emb_rows = vpad + P  # emb16 tensor rows: need ID_OFFSET..ID_OFFSET+vpad; keep simple pad

    nc = bacc.Bacc()
    D = {}
    def di(name, shape, dt):
        D[name] = nc.dram_tensor(name, shape, dt, kind="ExternalInput")
        return D[name]

    emb = di("emb", [VOCAB, HID], f32)
    cntT = di("cntT", [vpad, lpad], f16)
    nidrep = di("nidrep", [P, lpad], f16)
    eidx = di("eidx", [P, ncalls], i32)
    edst = di("edst", [P, ncalls], f32)
    gl = di("gl", [P, ntn], f32)
    growidx = di("growidx", [P, 1], i32)
    iotaV = di("iotaV", [P, nvc], f32)
    iota512 = di("iota512", [P, blk], f32)
    iota128 = di("iota128", [P, P], f32)
    idn64 = di("idn64", [HID, HID], f32)
    wpack = di("wpack", [HID, 9 * HID], f32)
    bsT = di("bsT", [HID, 9], f32)
    gammaT = di("gammaT", [HID, 3], f32)
    betaT = di("betaT", [HID, 3], f32)
    eps_rep = di("eps_rep", [P, 3], f32)
    woutp = di("woutp", [HID, 3 * NUM_CLASSES], f32)
    boutr = di("boutr", [P, NUM_CLASSES], f32)

    emb16 = nc.dram_tensor("emb16", [ID_OFFSET + vpad, HID], f16)
    hdev = [nc.dram_tensor(f"hdev{i}", [loc, HID], f16) for i in range(2)]
    hfull = [nc.dram_tensor(f"hfull{i}", [n, HID], f16, addr_space="Shared")
             for i in range(2)]
    statsin = [nc.dram_tensor(f"statsin{i}", [HID, 2], f32) for i in range(3)]
    statsout = [nc.dram_tensor(f"statsout{i}", [HID, 2], f32,
                               addr_space="Shared") for i in range(3)]
    obig = nc.dram_tensor("obig", [g + 1, NUM_CLASSES], f32)
    obig_red = nc.dram_tensor("obig_red", [g + 1, NUM_CLASSES], f32,
                              addr_space="Shared")
    out = nc.dram_tensor("out", [g, NUM_CLASSES], f32, kind="ExternalOutput")

    RG = [list(range(NCORES))]

    with tile.TileContext(nc) as tc:
        with (
            tc.tile_pool(name="c1", bufs=1) as c1,
            tc.tile_pool(name="cnt", bufs=3) as cntp,
            tc.tile_pool(name="rhs", bufs=3) as rhsp,
            tc.tile_pool(name="hs", bufs=64) as hsp,
            tc.tile_pool(name="oh", bufs=3) as ohp,
            tc.tile_pool(name="sb", bufs=4) as sbp,
            tc.tile_pool(name="rows", bufs=3) as rowsp,
            tc.tile_pool(name="psx", bufs=2, space="PSUM") as psx,
            tc.tile_pool(name="psm", bufs=2, space="PSUM") as psm,
            tc.tile_pool(name="pst", bufs=2, space="PSUM") as pst,
            tc.tile_pool(name="psp", bufs=1, space="PSUM") as psp,
        ):
            # ---- constants to SBUF ----
            def load(tname, dram, shape, dt):
                t = c1.tile(shape, dt, tag=tname)
                nc.sync.dma_start(out=t[:], in_=dram[:])
                return t
            eidx_sb = load("eidx", eidx, [P, ncalls], i32)
            edst_sb = load("edst", edst, [P, ncalls], f32)
            nid_sb = load("nidrep", nidrep, [P, lpad], f16)
            gl_sb = load("gl", gl, [P, ntn], f32)
            grow_sb = load("growidx", growidx, [P, 1], i32)
            iV_sb = load("iotaV", iotaV, [P, nvc], f32)
            i512_sb = load("iota512", iota512, [P, blk], f32)
            i128_sb = load("iota128", iota128, [P, P], f32)
            idn_sb = load("idn64", idn64, [HID, HID], f32)
            w_sb = load("wpack", wpack, [HID, 9 * HID], f32)
            b_sb = load("bsT", bsT, [HID, 9], f32)
            gam_sb = load("gammaT", gammaT, [HID, 3], f32)
            bet_sb = load("betaT", betaT, [HID, 3], f32)
            eps_sb = load("eps_rep", eps_rep, [P, 3], f32)
            wo_sb = load("woutp", woutp, [HID, 3 * NUM_CLASSES], f32)
            bo_sb = load("boutr", boutr, [P, NUM_CLASSES], f32)

            e1p32 = c1.tile([P, 3], f32)
            nc.vector.tensor_scalar(out=e1p32[:], in0=eps_sb[:], scalar1=1.0,
                                    scalar2=None, op0=AL.add)

            # ---- emb cast f32 -> f16 (rows ID_OFFSET.. only are used) ----
            zt16 = c1.tile([P, HID], f16)
            nc.vector.memset(zt16[:], 0.0)
            nrt = (VOCAB - ID_OFFSET + P - 1) // P   # chunks starting at ID_OFFSET
            for t in range(nrt):
                r0 = ID_OFFSET + t * P
                r1 = min(r0 + P, VOCAB)
                et = rowsp.tile([P, HID], f32, tag="embcast")
                nc.sync.dma_start(out=et[:r1 - r0, :], in_=emb[r0:r1, :])
                et16 = rowsp.tile([P, HID], f16, tag="embcast16")
                nc.vector.tensor_copy(out=et16[:r1 - r0, :], in_=et[:r1 - r0, :])
                if r1 - r0 < P:
                    nc.vector.tensor_copy(out=et16[r1 - r0:, :],
                                          in_=zt16[:P - (r1 - r0), :])
                nc.sync.dma_start(out=emb16[r0:r0 + P, :], in_=et16[:])
            # chunks fully beyond VOCAB (zero counts) -> zero rows
            for t in range(nrt, nvc):
                r0 = ID_OFFSET + t * P
                nc.sync.dma_start(out=emb16[r0:r0 + P, :], in_=zt16[:])

            embc = c1.tile([P, nvc * HID], f16)
            for vc in range(nvc):
                nc.sync.dma_start(
                    out=embc[:, vc * HID:(vc + 1) * HID],
                    in_=emb16[ID_OFFSET + vc * P:ID_OFFSET + (vc + 1) * P, :])

            hTown = c1.tile([HID, lpad], f32)
            x3f = c1.tile([HID, lpad], f32)
            pooledT = c1.tile([HID, 3 * P], f32)

            # ================= layers =================
            for l in range(3):
                for b in range(nblk):
                    bs0 = b * blk
                    bw = min(blk, lpad - bs0)
                    if l == 0:
                        ps_x = psx.tile([HID, blk], f32, space="PSUM", tag="psx")
                        for vc in range(nvc):
                            ct = cntp.tile([P, blk], f16)
                            nc.sync.dma_start(
                                out=ct[:, :bw],
                                in_=cntT[vc * P:(vc + 1) * P, bs0:bs0 + bw])
                            r2 = rhsp.tile([P, blk], f16)
                            nc.vector.tensor_scalar(
                                out=r2[:, :bw], in0=nid_sb[:, bs0:bs0 + bw],
                                scalar1=iV_sb[:, vc:vc + 1],
                                scalar2=e1p32[:, 0:1],
                                op0=AL.is_equal, op1=AL.mult)
                            nc.vector.tensor_tensor(
                                out=r2[:, :bw], in0=r2[:, :bw], in1=ct[:, :bw],
                                op=AL.add)
                            nc.tensor.matmul(
                                ps_x[:, :bw],
                                lhsT=embc[:, vc * HID:(vc + 1) * HID],
                                rhs=r2[:, :bw],
                                start=(vc == 0), stop=(vc == nvc - 1))
                        xT = sbp.tile([HID, blk], f32, tag="xT")
                        nc.vector.tensor_copy(out=xT[:, :bw], in_=ps_x[:, :bw])
                        xsrc = xT
                    else:
                        hprev = hfull[l - 1]
                        ps_a = psx.tile([HID, blk], f32, space="PSUM", tag="psx")
                        cal0 = b * tot_t
                        for t0 in range(0, tot_t, OHB):
                            nb = min(OHB, tot_t - t0)
                            oh = ohp.tile([P, OHB * blk], f16)
                            a0 = edst_sb[:, cal0 + t0:cal0 + t0 + nb]
                            in0 = bass.AP(a0.tensor, a0.offset,
                                          [a0.ap[0], [1, nb], [0, blk]])
                            a1 = i512_sb[:]
                            in1 = bass.AP(a1.tensor, a1.offset,
                                          [a1.ap[0], [0, nb], [1, blk]])
                            nc.vector.tensor_tensor(
                                out=oh[:, :nb * blk], in0=in0, in1=in1,
                                op=AL.is_equal)
                            for tt in range(nb):
                                t = t0 + tt
                                cal = cal0 + t
                                w = 0 if t < tiles_w[0] else 1
                                hs = hsp.tile([P, HID], f16)
                                nc.gpsimd.indirect_dma_start(
                                    out=hs[:], out_offset=None, in_=hprev[:],
                                    in_offset=bass.IndirectOffsetOnAxis(
                                        ap=eidx_sb[:, cal:cal + 1], axis=0),
                                    element_offset=(wr * HID if w == 1 else 0))
                                nc.tensor.matmul(
                                    ps_a[:, :bw], lhsT=hs[:],
                                    rhs=oh[:, tt * blk:tt * blk + bw],
                                    start=(t == 0), stop=(t == tot_t - 1))
                        xT = sbp.tile([HID, blk], f32, tag="xT")
                        nc.vector.tensor_scalar(
                            out=xT[:, :bw], in0=hTown[:, bs0:bs0 + bw],
                            scalar1=e1p32[:HID, l:l + 1], scalar2=None,
                            op0=AL.mult)
                        nc.vector.tensor_tensor(
                            out=xT[:, :bw], in0=xT[:, :bw], in1=ps_a[:, :bw],
                            op=AL.add)
                        xsrc = xT
                    cur = xsrc
                    for m in range(3):
                        ps_m = psm.tile([HID, blk], f32, space="PSUM", tag="psm")
                        nc.tensor.matmul(
                            ps_m[:, :bw],
                            lhsT=w_sb[:, (3 * l + m) * HID:(3 * l + m + 1) * HID],
                            rhs=cur[:, :bw],
                            start=True, stop=True)
                        if m == 2:
                            dst_t = x3f[:, bs0:bs0 + bw]
                        else:
                            nxt = sbp.tile([HID, blk], f32, tag="mlp")
                            dst_t = nxt[:, :bw]
                        nc.scalar.activation(
                            out=dst_t, in_=ps_m[:, :bw], func=AF.Relu,
                            bias=b_sb[:, 3 * l + m:3 * l + m + 1])
                        cur = dst_t if m == 2 else nxt

                # ---- BN stats + AllReduce ----
                st = sbp.tile([HID, 2], f32, tag="st")
                nc.vector.tensor_reduce(out=st[:, 0:1], in_=x3f[:, :loc],
                                        axis=mybir.AxisListType.X, op=AL.add)
                nc.scalar.activation(out=hTown[:, :loc], in_=x3f[:, :loc],
                                     func=AF.Square, accum_out=st[:, 1:2])
                nc.sync.dma_start(out=statsin[l][:], in_=st[:])
                nc.gpsimd.collective_compute(
                    "AllReduce", AL.add, replica_groups=RG,
                    ins=[statsin[l][:]], outs=[statsout[l][:]])
                rd = sbp.tile([HID, 2], f32, tag="rd")
                nc.sync.dma_start(out=rd[:], in_=statsout[l][:])
                mv = sbp.tile([HID, 1], f32, tag="mv")
                nc.vector.tensor_scalar(out=mv[:], in0=rd[:, 0:1],
                                        scalar1=1.0 / n, scalar2=None,
                                        op0=AL.mult)
                vr = sbp.tile([HID, 1], f32, tag="vr")
                nc.vector.tensor_scalar(out=vr[:], in0=rd[:, 1:2],
                                        scalar1=1.0 / n, scalar2=None,
                                        op0=AL.mult)
                m2 = sbp.tile([HID, 1], f32, tag="m2")
                nc.vector.tensor_tensor(out=m2[:], in0=mv[:], in1=mv[:],
                                        op=AL.mult)
                nc.vector.tensor_tensor(out=vr[:], in0=vr[:], in1=m2[:],
                                        op=AL.subtract)
                nc.vector.tensor_scalar(out=vr[:], in0=vr[:], scalar1=BN_EPS,
                                        scalar2=None, op0=AL.add)
                sq = sbp.tile([HID, 1], f32, tag="sq")
                nc.scalar.activation(out=sq[:], in_=vr[:], func=AF.Sqrt)
                inv = sbp.tile([HID, 1], f32, tag="inv")
                nc.vector.reciprocal(out=inv[:], in_=sq[:])
                scl = sbp.tile([HID, 1], f32, tag="scl")
                nc.vector.tensor_tensor(out=scl[:], in0=inv[:],
                                        in1=gam_sb[:, l:l + 1], op=AL.mult)
                shf = sbp.tile([HID, 1], f32, tag="shf")
                nc.vector.tensor_tensor(out=shf[:], in0=mv[:], in1=scl[:],
                                        op=AL.mult)
                nc.vector.tensor_tensor(out=shf[:], in0=bet_sb[:, l:l + 1],
                                        in1=shf[:], op=AL.subtract)
                # h_{l+1} = x3*scl + shf  (overwrites hTown)
                nc.vector.tensor_scalar(out=hTown[:], in0=x3f[:],
                                        scalar1=scl[:], scalar2=shf[:],
                                        op0=AL.mult, op1=AL.add)

                # ---- rows pass: transpose, pool, writeback ----
                ps_p = psp.tile([HID, P], f32, space="PSUM", tag="psp")
                for nt in range(ntn):
                    ps_t = pst.tile([P, HID], f32, space="PSUM", tag="pst")
                    nc.tensor.transpose(ps_t[:], hTown[:, nt * P:(nt + 1) * P],
                                        idn_sb[:])
                    r16 = rowsp.tile([P, HID], f16, tag="r16")
                    nc.scalar.activation(out=r16[:], in_=ps_t[:], func=AF.Copy)
                    if l < 2:
                        nr = min(P, loc - nt * P)
                        if nr > 0:
                            nc.sync.dma_start(out=hdev[l][nt * P:nt * P + nr, :],
                                              in_=r16[:nr, :])
                    gh = rowsp.tile([P, P], f16, tag="gh")
                    nc.vector.tensor_scalar(out=gh[:], in0=i128_sb[:],
                                            scalar1=gl_sb[:, nt:nt + 1],
                                            scalar2=None, op0=AL.is_equal)
                    nc.tensor.matmul(ps_p[:], lhsT=r16[:], rhs=gh[:],
                                     start=(nt == 0), stop=(nt == ntn - 1))
                nc.vector.tensor_copy(out=pooledT[:, l * P:(l + 1) * P],
                                      in_=ps_p[:])
                if l < 2:
                    nc.gpsimd.collective_compute(
                        "AllGather", AL.bypass, replica_groups=RG,
                        ins=[hdev[l][:]], outs=[hfull[l][:]])

            # ================= final linear =================
            ps_o = psp.tile([P, NUM_CLASSES], f32, space="PSUM", tag="psp")
            for l in range(3):
                nc.tensor.matmul(
                    ps_o[:], lhsT=pooledT[:, l * P:(l + 1) * P],
                    rhs=wo_sb[:, l * NUM_CLASSES:(l + 1) * NUM_CLASSES],
                    start=(l == 0), stop=(l == 2))
            oloc = sbp.tile([P, NUM_CLASSES], f32, tag="oloc")
            nc.vector.tensor_copy(out=oloc[:], in_=ps_o[:])
            zt = sbp.tile([P, NUM_CLASSES], f32, tag="zt")
            nc.vector.memset(zt[:], 0.0)
            nzt = (g + 1 + P - 1) // P
            for r in range(nzt):
                r0 = r * P
                r1 = min(r0 + P, g + 1)
                nc.sync.dma_start(out=obig[r0:r1, :], in_=zt[:r1 - r0, :])
            nc.gpsimd.indirect_dma_start(
                out=obig[:], out_offset=bass.IndirectOffsetOnAxis(
                    ap=grow_sb[:, 0:1], axis=0),
                in_=oloc[:], in_offset=None)
            nc.gpsimd.collective_compute(
                "AllReduce", AL.add, replica_groups=RG,
                ins=[obig[:]], outs=[obig_red[:]])
            for r in range((g + P - 1) // P):
                r0 = r * P
                r1 = min(r0 + P, g)
                ot = sbp.tile([P, NUM_CLASSES], f32, tag="ot")
                nc.sync.dma_start(out=ot[:r1 - r0, :], in_=obig_red[r0:r1, :])
                nc.vector.tensor_tensor(out=ot[:r1 - r0, :], in0=ot[:r1 - r0, :],
                                        in1=bo_sb[:r1 - r0, :], op=AL.add)
                nc.sync.dma_start(out=out[r0:r1, :], in_=ot[:r1 - r0, :])
    return nc


def kernel(**inputs):
    global _LAST_EXEC_NS
    import concourse.bass_utils as bass_utils
    bass_utils.upload_artifacts = lambda tmpdir: tmpdir
    from concourse.bass_utils import run_bass_kernel_spmd

    cfg = CFG
    per_core, tiles_w, ncalls = _prep(
        cfg, inputs["node_ids"], inputs["edge_src"], inputs["edge_dst"],
        inputs["graph_ids"], inputs["Ws"], inputs["bs"], inputs["bn_gamma"],
        inputs["bn_beta"], inputs["eps"], inputs["W_out"], inputs["b_out"],
        inputs["emb"])
    nc = _build(cfg, tiles_w, ncalls)
    nc.finalize()
    trace = bool(_PROFILE) and _install_profile_hook()
    res = run_bass_kernel_spmd(nc, per_core, core_ids=list(range(NCORES)),
                               trace=trace)
    _LAST_EXEC_NS = res.exec_time_ns
    return np.asarray(res.results[0]["out"], np.float32)



# revision 15
# speedup vs baseline: 1.3266x; 1.3266x over previous
"""TRN2 Bass kernel: 3-layer GIN (sum-agg) + MLP + BatchNorm + graph sum-pooling + linear.

Full inputs in, full output out. Internally: 8-way data parallel over nodes
(12500 contiguous nodes/core), SPMD NEFF via run_bass_kernel_spmd.

Aggregation strategy (per core, per layer):
  l==0: x factors through the 1600-wide vocabulary: counts matmul (no gathers).
  l>0 : batched SWDGE dma_gather pulls up to 1024 h rows per call (the ~1us
        SWDGE fixed cost amortized 8x vs 128-row indirect DMA), from an fp16
        table padded to 128 cols (256B rows, the gather granularity), int16
        indices windowed into 32768-row ranges, wrapped into 16 partitions
        and replicated across the 8 Q7 cores.
        Scatter into PSUM via one-hot matmuls: first tile of each 512-col dst
        block is 512 wide with start=True (zeroes the bank), subsequent tiles
        are 128-col windows at fixed 128-aligned bin offsets with start=False.
  Then 3x(Linear+ReLU) on PE/ACT, BN stats cross-core via AllReduce, per-graph
  pooling via one-hot matmul, h writeback (PE transpose -> fp16 rows) +
  AllGather.
Final: pooledT_l @ W_out slices accumulate -> [128,100], indirect-scatter into
[513,100] by graph window, AllReduce over cores, + b_out.
"""
import sys
import types

import numpy as np

HID = 64
P = 128
NCORES = 8
VOCAB = 3100
ID_OFFSET = 1500
NUM_CLASSES = 100
BN_EPS = 1e-5

CFG = dict(
    n_nodes=100_000,
    n_graphs=512,
    win_rows=32_768,   # int16 index reach per gather window
    blk=512,           # dst nodes per PSUM block (one f32 bank)
    call_tiles=8,      # max 128-row tiles per dma_gather call (HW cap: 1024 idxs)
    oh_batch=8,        # narrow one-hot tiles built per DVE op
)

_PROFILE = False
_LAST_EXEC_NS = None


def _install_profile_hook():
    try:
        import antenv
        from trn_agent_boot.trn_boot import _ntff_profile_via_ctypes
    except Exception:
        return False
    if "antenv.axon_hooks" in sys.modules:
        return True
    hooks = types.ModuleType("antenv.axon_hooks")
    hooks._hook = _ntff_profile_via_ctypes("/opt/axon/libaxon_pjrt.so")
    hooks.set_axon_ntff_profile_hook = lambda h: setattr(hooks, "_hook", h)
    hooks.get_axon_ntff_profile_hook = lambda: hooks._hook
    sys.modules["antenv.axon_hooks"] = hooks
    antenv.axon_hooks = hooks
    return True


def _derived(cfg):
    n, blk = cfg["n_nodes"], cfg["blk"]
    loc = n // NCORES
    lpad = ((loc + blk - 1) // blk) * blk
    nblk = lpad // blk
    ntn = lpad // P
    vpad = ((VOCAB - ID_OFFSET + P - 1) // P) * P   # 1664
    nvc = vpad // P
    nwin = (n + cfg["win_rows"] - 1) // cfg["win_rows"]
    nbin = blk // P
    return loc, lpad, nblk, ntn, vpad, nvc, nwin, nbin


class Sched:
    """Compile-time (core-independent) gather/scatter schedule."""
    pass


def _schedule(cfg, edge_src, edge_dst):
    """Build shared schedule from per-(block,window,bin) caps across cores."""
    loc, lpad, nblk, ntn, vpad, nvc, nwin, nbin = _derived(cfg)
    n, wr, blk = cfg["n_nodes"], cfg["win_rows"], cfg["blk"]
    MAXT = cfg["call_tiles"]
    ngrp = nblk * nwin * nbin

    cnts = np.zeros((NCORES, ngrp), np.int64)
    for c in range(NCORES):
        base = c * loc
        m = (edge_dst >= base) & (edge_dst < base + loc)
        src = edge_src[m]
        dl = edge_dst[m] - base
        b = dl // blk
        w = src // wr
        bn = (dl % blk) // P
        key = (b * nwin + w) * nbin + bn
        cnts[c] = np.bincount(key, minlength=ngrp)

    cap = ((cnts.max(axis=0) + P - 1) // P) * P  # [ngrp]
    cap = cap.reshape(nblk, nwin, nbin)
    for b in range(nblk):
        if cap[b].sum() == 0:
            cap[b, 0, 0] = P  # pad tile so the block's PSUM bank gets started

    # tile order: (block, window, bin); calls chop at window changes / MAXT
    s = Sched()
    s.grp_slot0 = {}          # (b, w, bn) -> slot0
    s.tiles = []              # (block, col0, width, start, stop, win)
    slot0 = 0
    first_t, last_t = {}, {}
    for b in range(nblk):
        for w in range(nwin):
            for bn in range(nbin):
                cp = int(cap[b, w, bn])
                if cp == 0:
                    continue
                s.grp_slot0[(b, w, bn)] = slot0
                for _t in range(cp // P):
                    ti = len(s.tiles)
                    if b not in first_t:
                        first_t[b] = ti
                    last_t[b] = ti
                    s.tiles.append([b, bn * P, P, False, False, w])
                    slot0 += P
    for b, ti in first_t.items():
        s.tiles[ti][1] = 0
        s.tiles[ti][2] = blk
        s.tiles[ti][3] = True
    for b, ti in last_t.items():
        s.tiles[ti][4] = True

    s.calls = []              # dict(win, tile0, ntiles)
    t0 = 0
    for t in range(len(s.tiles) + 1):
        flush = (t == len(s.tiles) or t - t0 == MAXT
                 or (t > t0 and s.tiles[t][5] != s.tiles[t0][5]))
        if flush and t > t0:
            s.calls.append(dict(win=s.tiles[t0][5], tile0=t0, ntiles=t - t0))
            t0 = t
    s.nslots = slot0
    s.ntiles = len(s.tiles)
    s.cap = cap
    s.max_call_tiles = max(c["ntiles"] for c in s.calls)
    return s


def _prep(cfg, s, node_ids, edge_src, edge_dst, graph_ids, Ws, bs, bn_gamma,
          bn_beta, eps, W_out, b_out, emb):
    """Index-only host preprocessing -> per-core input dicts."""
    loc, lpad, nblk, ntn, vpad, nvc, nwin, nbin = _derived(cfg)
    n, wr, blk = cfg["n_nodes"], cfg["win_rows"], cfg["blk"]
    node_ids = np.asarray(node_ids, np.int64)
    edge_src = np.asarray(edge_src, np.int64)
    edge_dst = np.asarray(edge_dst, np.int64)
    graph_ids = np.asarray(graph_ids, np.int64)

    tile_block = np.array([t[0] for t in s.tiles], np.int64)
    tile_col0 = np.array([t[1] for t in s.tiles], np.int64)
    tile_wide = np.array([t[2] == blk for t in s.tiles], bool)

    per_core = []
    for c in range(NCORES):
        base = c * loc
        m = (edge_dst >= base) & (edge_dst < base + loc)
        src = edge_src[m]
        dl = edge_dst[m] - base
        b = dl // blk
        w = src // wr
        bn = (dl % blk) // P

        # slot assignment: edges of group g sorted by dl, packed from grp_slot0
        gkey = (b * nwin + w) * nbin + bn
        order = np.lexsort((dl, gkey))
        src, dl, w, gkey = (a[order] for a in (src, dl, w, gkey))
        slots = np.empty(len(src), np.int64)
        gs, gidx = np.unique(gkey, return_index=True)
        bounds = list(gidx) + [len(src)]
        for gi, g in enumerate(gs):
            lo, hi = bounds[gi], bounds[gi + 1]
            bb = int(g) // (nwin * nbin)
            ww = (int(g) // nbin) % nwin
            bnn = int(g) % nbin
            s0 = s.grp_slot0[(bb, ww, bnn)]
            slots[lo:hi] = s0 + np.arange(hi - lo)

        idx_flat = np.zeros(s.nslots, np.int16)
        idx_flat[slots] = (src - w * wr).astype(np.int16)
        edst_flat = np.full(s.nslots, -1.0, np.float32)
        tl = slots // P
        colbase = np.where(tile_wide[tl], 0, tile_col0[tl])
        edst_flat[slots] = (dl - tile_block[tl] * blk - colbase).astype(np.float32)

        sl = np.arange(s.nslots)
        # gather idx layout: slot i -> (partition i%16, col i//16), replicated
        # across the 8 Q7 cores (16-partition groups)
        idx_w = np.zeros((16, s.nslots // 16), np.int16)
        idx_w[sl % 16, sl // 16] = idx_flat
        idx_w = np.tile(idx_w, (8, 1))
        edst_t = np.zeros((P, s.ntiles), np.float32)
        edst_t[sl % P, sl // P] = edst_flat

        # vocab count matrix cntT [vpad, lpad] fp16
        v = node_ids[src]
        cnt = np.bincount(v * lpad + dl, minlength=vpad * lpad)
        cntT = cnt.reshape(vpad, lpad).astype(np.float16)

        nidrep = np.full((P, lpad), -1.0, np.float16)
        nidrep[:, :loc] = np.tile(node_ids[base:base + loc].astype(np.float16),
                                  (P, 1))
        gl = np.full((P, ntn), -1.0, np.float32)
        g_base = int(graph_ids[base])
        g_span = int(graph_ids[base + loc - 1]) - g_base
        assert g_span < P, f"graph window {g_span} >= {P}"
        glv = (graph_ids[base:base + loc] - g_base).astype(np.float32)
        gl_full = np.full(lpad, -1.0, np.float32)
        gl_full[:loc] = glv
        gl[:, :] = gl_full.reshape(ntn, P).T
        growidx = np.minimum(g_base + np.arange(P), cfg["n_graphs"]).astype(
            np.int32)[:, None]

        wpack = np.zeros((HID, 9 * HID), np.float32)
        for l in range(3):
            for mm in range(3):
                wpack[:, (3 * l + mm) * HID:(3 * l + mm + 1) * HID] = Ws[l, mm]
        woutp = np.zeros((HID, 3 * NUM_CLASSES), np.float32)
        for l in range(3):
            woutp[:, l * NUM_CLASSES:(l + 1) * NUM_CLASSES] = \
                W_out[l * HID:(l + 1) * HID]

        per_core.append(dict(
            emb=np.asarray(emb, np.float32),
            cntT=cntT,
            nidrep=nidrep,
            idxw=idx_w,
            edstt=edst_t,
            gl=gl,
            growidx=growidx,
            iotaV=(np.arange(P, dtype=np.float32)[:, None]
                   + P * np.arange(nvc, dtype=np.float32)[None, :]),
            iota512=np.tile(np.arange(blk, dtype=np.float32)[None, :], (P, 1)),
            iota128=np.tile(np.arange(P, dtype=np.float32)[None, :], (P, 1)),
            idn64=np.eye(HID, dtype=np.float16),
            wpack=wpack,
            bsT=np.asarray(bs, np.float32).reshape(9, HID).T.copy(),
            gammaT=np.asarray(bn_gamma, np.float32).T.copy(),
            betaT=np.asarray(bn_beta, np.float32).T.copy(),
            eps_rep=np.tile(np.asarray(eps, np.float32)[None, :], (P, 1)),
            woutp=woutp,
            boutr=np.tile(np.asarray(b_out, np.float32)[None, :], (P, 1)),
        ))
    return per_core


def _build(cfg, s):
    import concourse.bacc as bacc
    import concourse.bass as bass
    import concourse.mybir as mybir
    import concourse.tile as tile

    loc, lpad, nblk, ntn, vpad, nvc, nwin, nbin = _derived(cfg)
    n, g, wr, blk = (cfg["n_nodes"], cfg["n_graphs"], cfg["win_rows"],
                     cfg["blk"])
    OHB = cfg["oh_batch"]
    f32, f16, i32 = mybir.dt.float32, mybir.dt.float16, mybir.dt.int32
    i16 = mybir.dt.int16
    AL, AF = mybir.AluOpType, mybir.ActivationFunctionType

    nc = bacc.Bacc()
    D = {}
    def di(name, shape, dt):
        D[name] = nc.dram_tensor(name, shape, dt, kind="ExternalInput")
        return D[name]

    emb = di("emb", [VOCAB, HID], f32)
    cntT = di("cntT", [vpad, lpad], f16)
    nidrep = di("nidrep", [P, lpad], f16)
    idxw = di("idxw", [P, s.nslots // 16], i16)
    edstt = di("edstt", [P, s.ntiles], f32)
    gl = di("gl", [P, ntn], f32)
    growidx = di("growidx", [P, 1], i32)
    iotaV = di("iotaV", [P, nvc], f32)
    iota512 = di("iota512", [P, blk], f32)
    iota128 = di("iota128", [P, P], f32)
    idn64 = di("idn64", [HID, HID], f16)
    wpack = di("wpack", [HID, 9 * HID], f32)
    bsT = di("bsT", [HID, 9], f32)
    gammaT = di("gammaT", [HID, 3], f32)
    betaT = di("betaT", [HID, 3], f32)
    eps_rep = di("eps_rep", [P, 3], f32)
    woutp = di("woutp", [HID, 3 * NUM_CLASSES], f32)
    boutr = di("boutr", [P, NUM_CLASSES], f32)

    hdev = [nc.dram_tensor(f"hdev{i}", [loc, P], f16) for i in range(2)]
    hfull = [nc.dram_tensor(f"hfull{i}", [n, P], f16, addr_space="Shared")
             for i in range(2)]
    statsin = [nc.dram_tensor(f"statsin{i}", [HID, 2], f32) for i in range(3)]
    statsout = [nc.dram_tensor(f"statsout{i}", [HID, 2], f32,
                               addr_space="Shared") for i in range(3)]
    obig = nc.dram_tensor("obig", [g + 1, NUM_CLASSES], f32)
    obig_red = nc.dram_tensor("obig_red", [g + 1, NUM_CLASSES], f32,
                              addr_space="Shared")
    out = nc.dram_tensor("out", [g, NUM_CLASSES], f32, kind="ExternalOutput")

    RG = [list(range(NCORES))]

    with tile.TileContext(nc) as tc:
        with (
            tc.tile_pool(name="c1", bufs=1) as c1,
            tc.tile_pool(name="cnt", bufs=3) as cntp,
            tc.tile_pool(name="rhs", bufs=3) as rhsp,
            tc.tile_pool(name="gat", bufs=3) as gatp,
            tc.tile_pool(name="idxp", bufs=3) as idxp,
            tc.tile_pool(name="oh", bufs=3) as ohp,
            tc.tile_pool(name="sb", bufs=4) as sbp,
            tc.tile_pool(name="rows", bufs=3) as rowsp,
            tc.tile_pool(name="psx", bufs=3, space="PSUM") as psx,
            tc.tile_pool(name="psm", bufs=2, space="PSUM") as psm,
            tc.tile_pool(name="pst", bufs=2, space="PSUM") as pst,
            tc.tile_pool(name="psp", bufs=1, space="PSUM") as psp,
        ):
            # ---- constants to SBUF ----
            def load(tname, dram, shape, dt):
                t = c1.tile(shape, dt, tag=tname)
                nc.sync.dma_start(out=t[:], in_=dram[:])
                return t
            nid_sb = load("nidrep", nidrep, [P, lpad], f16)
            edst_sb = load("edstt", edstt, [P, s.ntiles], f32)
            gl_sb = load("gl", gl, [P, ntn], f32)
            grow_sb = load("growidx", growidx, [P, 1], i32)
            iV_sb = load("iotaV", iotaV, [P, nvc], f32)
            i512_sb = load("iota512", iota512, [P, blk], f32)
            i128_sb = load("iota128", iota128, [P, P], f32)
            idn_sb = load("idn64", idn64, [HID, HID], f16)
            w_sb = load("wpack", wpack, [HID, 9 * HID], f32)
            b_sb = load("bsT", bsT, [HID, 9], f32)
            gam_sb = load("gammaT", gammaT, [HID, 3], f32)
            bet_sb = load("betaT", betaT, [HID, 3], f32)
            eps_sb = load("eps_rep", eps_rep, [P, 3], f32)
            wo_sb = load("woutp", woutp, [HID, 3 * NUM_CLASSES], f32)
            bo_sb = load("boutr", boutr, [P, NUM_CLASSES], f32)

            e1p32 = c1.tile([P, 3], f32)
            nc.vector.tensor_scalar(out=e1p32[:], in0=eps_sb[:], scalar1=1.0,
                                    scalar2=None, op0=AL.add)

            # ---- emb -> fp16 vocab chunks in SBUF [P, nvc*HID] ----
            embc = c1.tile([P, nvc * HID], f16)
            zt16 = c1.tile([P, HID], f16)
            nc.vector.memset(zt16[:], 0.0)
            for vc in range(nvc):
                r0 = ID_OFFSET + vc * P
                r1 = min(r0 + P, VOCAB)
                if r1 > r0:
                    et = rowsp.tile([P, HID], f32, tag="embcast")
                    nc.sync.dma_start(out=et[:r1 - r0, :], in_=emb[r0:r1, :])
                    nc.vector.tensor_copy(out=embc[:r1 - r0,
                                                   vc * HID:(vc + 1) * HID],
                                          in_=et[:r1 - r0, :])
                    if r1 - r0 < P:
                        nc.vector.tensor_copy(
                            out=embc[r1 - r0:, vc * HID:(vc + 1) * HID],
                            in_=zt16[:P - (r1 - r0), :])
                else:
                    nc.vector.tensor_copy(out=embc[:, vc * HID:(vc + 1) * HID],
                                          in_=zt16[:])

            hTown = c1.tile([HID, lpad], f16)
            x3f = c1.tile([HID, lpad], f16)
            pooledT = c1.tile([HID, 3 * P], f32)

            def mlp_block(l, cur, bs0):
                """3x(Linear+ReLU) on feature-major block; writes x3f slice."""
                for m in range(3):
                    ps_m = psm.tile([HID, blk], f32, space="PSUM", tag="psm")
                    nc.tensor.matmul(
                        ps_m[:],
                        lhsT=w_sb[:, (3 * l + m) * HID:(3 * l + m + 1) * HID],
                        rhs=cur, start=True, stop=True)
                    if m == 2:
                        dst_t = x3f[:, bs0:bs0 + blk]
                    else:
                        nxt = sbp.tile([HID, blk], f32, tag="mlp")
                        dst_t = nxt[:]
                    nc.scalar.activation(
                        out=dst_t, in_=ps_m[:], func=AF.Relu,
                        bias=b_sb[:, 3 * l + m:3 * l + m + 1])
                    cur = dst_t if m == 2 else nxt[:]

            # ================= layers =================
            for l in range(3):
                if l == 0:
                    for b in range(nblk):
                        bs0 = b * blk
                        ps_x = psx.tile([HID, blk], f32, space="PSUM",
                                        tag="psx")
                        for vc in range(nvc):
                            ct = cntp.tile([P, blk], f16)
                            nc.sync.dma_start(
                                out=ct[:],
                                in_=cntT[vc * P:(vc + 1) * P, bs0:bs0 + blk])
                            r2 = rhsp.tile([P, blk], f16)
                            nc.vector.tensor_scalar(
                                out=r2[:], in0=nid_sb[:, bs0:bs0 + blk],
                                scalar1=iV_sb[:, vc:vc + 1],
                                scalar2=e1p32[:, 0:1],
                                op0=AL.is_equal, op1=AL.mult)
                            nc.vector.tensor_tensor(
                                out=r2[:], in0=r2[:], in1=ct[:], op=AL.add)
                            nc.tensor.matmul(
                                ps_x[:],
                                lhsT=embc[:, vc * HID:(vc + 1) * HID],
                                rhs=r2[:],
                                start=(vc == 0), stop=(vc == nvc - 1))
                        xT = sbp.tile([HID, blk], f32, tag="xT")
                        nc.vector.tensor_copy(out=xT[:], in_=ps_x[:])
                        mlp_block(l, xT[:], bs0)
                else:
                    hprev = hfull[l - 1]
                    live_ps = {}
                    for call in s.calls:
                        w = call["win"]
                        w0 = w * wr
                        w1 = min(w0 + wr, n)
                        t0c, nt_c = call["tile0"], call["ntiles"]
                        ns_c = nt_c * P
                        it = idxp.tile([P, s.max_call_tiles * P // 16], i16,
                                       tag="idx")
                        nc.sync.dma_start(
                            out=it[:, :ns_c // 16],
                            in_=idxw[:, t0c * P // 16:
                                     (t0c * P + ns_c) // 16])
                        gt = gatp.tile([P, s.max_call_tiles * P], f16,
                                       tag="gat")
                        g0 = gt[:]
                        gat3 = bass.AP(g0.tensor, g0.offset,
                                       [g0.ap[0], [P, nt_c], [1, P]])
                        nc.gpsimd.dma_gather(
                            out_ap=gat3,
                            in_ap=hprev[w0:w1, :],
                            idxs_ap=it[:, :ns_c // 16],
                            num_idxs=ns_c,
                            num_idxs_reg=ns_c,
                            elem_size=P)

                        ti = 0
                        while ti < nt_c:
                            t = t0c + ti
                            blk_t, col0, wdt, st, sp = s.tiles[t][:5]
                            if st:
                                ps_a = psx.tile([HID, blk], f32, space="PSUM",
                                                tag="psx")
                                live_ps[blk_t] = ps_a
                                oh = ohp.tile([P, OHB * P], f16, tag="oh")
                                assert blk <= OHB * P
                                a0 = edst_sb[:, t:t + 1]
                                in0 = bass.AP(a0.tensor, a0.offset,
                                              [a0.ap[0], [0, blk]])
                                nc.vector.tensor_tensor(
                                    out=oh[:, :blk], in0=in0, in1=i512_sb[:],
                                    op=AL.is_equal)
                                nc.tensor.matmul(
                                    ps_a[:],
                                    lhsT=gt[:, ti * P:ti * P + HID],
                                    rhs=oh[:, :blk], start=True, stop=sp)
                                nb = 1
                            else:
                                nb = 1
                                while (nb < OHB and ti + nb < nt_c
                                       and not s.tiles[t0c + ti + nb][3]):
                                    nb += 1
                                oh = ohp.tile([P, OHB * P], f16, tag="oh")
                                a0 = edst_sb[:, t:t + nb]
                                in0 = bass.AP(a0.tensor, a0.offset,
                                              [a0.ap[0], [1, nb], [0, P]])
                                a1 = i128_sb[:]
                                in1 = bass.AP(a1.tensor, a1.offset,
                                              [a1.ap[0], [0, nb], [1, P]])
                                nc.vector.tensor_tensor(
                                    out=oh[:, :nb * P], in0=in0, in1=in1,
                                    op=AL.is_equal)
                                for k in range(nb):
                                    tk = t0c + ti + k
                                    bk, c0k, wk, stk, spk = s.tiles[tk][:5]
                                    ps_a = live_ps[bk]
                                    nc.tensor.matmul(
                                        ps_a[:, c0k:c0k + P],
                                        lhsT=gt[:, (ti + k) * P:
                                                (ti + k) * P + HID],
                                        rhs=oh[:, k * P:(k + 1) * P],
                                        start=False, stop=spk)
                            # drain finished blocks
                            for k in range(nb):
                                tk = t0c + ti + k
                                bk = s.tiles[tk][0]
                                if s.tiles[tk][4]:
                                    ps_a = live_ps.pop(bk)
                                    bs0 = bk * blk
                                    xT = sbp.tile([HID, blk], f32, tag="xT")
                                    nc.vector.tensor_scalar(
                                        out=xT[:], in0=hTown[:, bs0:bs0 + blk],
                                        scalar1=e1p32[:HID, l:l + 1],
                                        scalar2=None, op0=AL.mult)
                                    nc.vector.tensor_tensor(
                                        out=xT[:], in0=xT[:], in1=ps_a[:],
                                        op=AL.add)
                                    mlp_block(l, xT[:], bs0)
                            ti += nb
                    assert not live_ps, f"unfinished blocks {list(live_ps)}"

                # ---- BN stats + AllReduce ----
                st_t = sbp.tile([HID, 2], f32, tag="st")
                nc.vector.tensor_reduce(out=st_t[:, 0:1], in_=x3f[:, :loc],
                                        axis=mybir.AxisListType.X, op=AL.add)
                nc.scalar.activation(out=hTown[:, :loc], in_=x3f[:, :loc],
                                     func=AF.Square, accum_out=st_t[:, 1:2])
                nc.sync.dma_start(out=statsin[l][:], in_=st_t[:])
                nc.gpsimd.collective_compute(
                    "AllReduce", AL.add, replica_groups=RG,
                    ins=[statsin[l][:]], outs=[statsout[l][:]])
                rd = sbp.tile([HID, 2], f32, tag="rd")
                nc.sync.dma_start(out=rd[:], in_=statsout[l][:])
                mv = sbp.tile([HID, 1], f32, tag="mv")
                nc.vector.tensor_scalar(out=mv[:], in0=rd[:, 0:1],
                                        scalar1=1.0 / n, scalar2=None,
                                        op0=AL.mult)
                vr = sbp.tile([HID, 1], f32, tag="vr")
                nc.vector.tensor_scalar(out=vr[:], in0=rd[:, 1:2],
                                        scalar1=1.0 / n, scalar2=None,
                                        op0=AL.mult)
                m2 = sbp.tile([HID, 1], f32, tag="m2")
                nc.vector.tensor_tensor(out=m2[:], in0=mv[:], in1=mv[:],
                                        op=AL.mult)
                nc.vector.tensor_tensor(out=vr[:], in0=vr[:], in1=m2[:],
                                        op=AL.subtract)
                nc.vector.tensor_scalar(out=vr[:], in0=vr[:], scalar1=BN_EPS,
                                        scalar2=None, op0=AL.add)
                sq = sbp.tile([HID, 1], f32, tag="sq")
                nc.scalar.activation(out=sq[:], in_=vr[:], func=AF.Sqrt)
                inv = sbp.tile([HID, 1], f32, tag="inv")
                nc.vector.reciprocal(out=inv[:], in_=sq[:])
                scl = sbp.tile([HID, 1], f32, tag="scl")
                nc.vector.tensor_tensor(out=scl[:], in0=inv[:],
                                        in1=gam_sb[:, l:l + 1], op=AL.mult)
                shf = sbp.tile([HID, 1], f32, tag="shf")
                nc.vector.tensor_tensor(out=shf[:], in0=mv[:], in1=scl[:],
                                        op=AL.mult)
                nc.vector.tensor_tensor(out=shf[:], in0=bet_sb[:, l:l + 1],
                                        in1=shf[:], op=AL.subtract)
                # h_{l+1} = x3*scl + shf  (overwrites hTown)
                nc.vector.tensor_scalar(out=hTown[:], in0=x3f[:],
                                        scalar1=scl[:], scalar2=shf[:],
                                        op0=AL.mult, op1=AL.add)

                # ---- rows pass: transpose, pool, writeback ----
                ps_p = psp.tile([HID, P], f32, space="PSUM", tag="psp")
                for nt in range(ntn):
                    ps_t = pst.tile([P, HID], f16, space="PSUM", tag="pst")
                    nc.tensor.transpose(ps_t[:], hTown[:, nt * P:(nt + 1) * P],
                                        idn_sb[:])
                    r16 = rowsp.tile([P, HID], f16, tag="r16")
                    nc.scalar.activation(out=r16[:], in_=ps_t[:], func=AF.Copy)
                    if l < 2:
                        nr = min(P, loc - nt * P)
                        if nr > 0:
                            nc.sync.dma_start(
                                out=hdev[l][nt * P:nt * P + nr, 0:HID],
                                in_=r16[:nr, :])
                    gh = rowsp.tile([P, P], f16, tag="gh")
                    nc.vector.tensor_scalar(out=gh[:], in0=i128_sb[:],
                                            scalar1=gl_sb[:, nt:nt + 1],
                                            scalar2=None, op0=AL.is_equal)
                    nc.tensor.matmul(ps_p[:], lhsT=r16[:], rhs=gh[:],
                                     start=(nt == 0), stop=(nt == ntn - 1))
                nc.vector.tensor_copy(out=pooledT[:, l * P:(l + 1) * P],
                                      in_=ps_p[:])
                if l < 2:
                    nc.gpsimd.collective_compute(
                        "AllGather", AL.bypass, replica_groups=RG,
                        ins=[hdev[l][:]], outs=[hfull[l][:]])

            # ================= final linear =================
            ps_o = psp.tile([P, NUM_CLASSES], f32, space="PSUM", tag="psp")
            for l in range(3):
                nc.tensor.matmul(
                    ps_o[:], lhsT=pooledT[:, l * P:(l + 1) * P],
                    rhs=wo_sb[:, l * NUM_CLASSES:(l + 1) * NUM_CLASSES],
                    start=(l == 0), stop=(l == 2))
            oloc = sbp.tile([P, NUM_CLASSES], f32, tag="oloc")
            nc.vector.tensor_copy(out=oloc[:], in_=ps_o[:])
            zt = sbp.tile([P, NUM_CLASSES], f32, tag="zt")
            nc.vector.memset(zt[:], 0.0)
            nzt = (g + 1 + P - 1) // P
            for r in range(nzt):
                r0 = r * P
                r1 = min(r0 + P, g + 1)
                nc.sync.dma_start(out=obig[r0:r1, :], in_=zt[:r1 - r0, :])
            nc.gpsimd.indirect_dma_start(
                out=obig[:], out_offset=bass.IndirectOffsetOnAxis(
                    ap=grow_sb[:, 0:1], axis=0),
                in_=oloc[:], in_offset=None)
            nc.gpsimd.collective_compute(
                "AllReduce", AL.add, replica_groups=RG,
                ins=[obig[:]], outs=[obig_red[:]])
            for r in range((g + P - 1) // P):
                r0 = r * P
                r1 = min(r0 + P, g)
                ot = sbp.tile([P, NUM_CLASSES], f32, tag="ot")
                nc.sync.dma_start(out=ot[:r1 - r0, :], in_=obig_red[r0:r1, :])
                nc.vector.tensor_tensor(out=ot[:r1 - r0, :], in0=ot[:r1 - r0, :],
                                        in1=bo_sb[:r1 - r0, :], op=AL.add)
                nc.sync.dma_start(out=out[r0:r1, :], in_=ot[:r1 - r0, :])
    return nc


def build_all(cfg, inputs):
    s = _schedule(cfg, np.asarray(inputs["edge_src"], np.int64),
                  np.asarray(inputs["edge_dst"], np.int64))
    per_core = _prep(
        cfg, s, inputs["node_ids"], inputs["edge_src"], inputs["edge_dst"],
        inputs["graph_ids"], inputs["Ws"], inputs["bs"], inputs["bn_gamma"],
        inputs["bn_beta"], inputs["eps"], inputs["W_out"], inputs["b_out"],
        inputs["emb"])
    nc = _build(cfg, s)
    return nc, per_core


def kernel(**inputs):
    global _LAST_EXEC_NS
    import concourse.bass_utils as bass_utils
    bass_utils.upload_artifacts = lambda tmpdir: tmpdir
    from concourse.bass_utils import run_bass_kernel_spmd

    nc, per_core = build_all(CFG, inputs)
    nc.finalize()
    trace = bool(_PROFILE) and _install_profile_hook()
    res = run_bass_kernel_spmd(nc, per_core, core_ids=list(range(NCORES)),
                               trace=trace)
    _LAST_EXEC_NS = res.exec_time_ns
    return np.asarray(res.results[0]["out"], np.float32)


# revision 16
# speedup vs baseline: 1.3928x; 1.0499x over previous
"""TRN2 Bass kernel: 3-layer GIN (sum-agg) + MLP + BatchNorm + graph sum-pooling + linear.

Full inputs in, full output out. Internally: 8-way data parallel over nodes
(12500 contiguous nodes/core), SPMD NEFF via run_bass_kernel_spmd.

Aggregation strategy (per core, per layer):
  l==0: x factors through the 1600-wide vocabulary: counts matmul (no gathers).
  l>0 : batched SWDGE dma_gather pulls up to 1024 h rows per call (the ~1us
        SWDGE fixed cost amortized 8x vs 128-row indirect DMA), from an fp16
        table padded to 128 cols (256B rows, the gather granularity), int16
        indices windowed into 32768-row ranges, wrapped into 16 partitions
        and replicated across the 8 Q7 cores.
        Scatter into PSUM via one-hot matmuls: first tile of each 512-col dst
        block is 512 wide with start=True (zeroes the bank), subsequent tiles
        are 128-col windows at fixed 128-aligned bin offsets with start=False.
  Then 3x(Linear+ReLU) on PE/ACT, BN stats cross-core via AllReduce, per-graph
  pooling via one-hot matmul, h writeback (PE transpose -> fp16 rows) +
  AllGather.
Final: pooledT_l @ W_out slices accumulate -> [128,100], indirect-scatter into
[513,100] by graph window, AllReduce over cores, + b_out.
"""
import sys
import types

import numpy as np

HID = 64
P = 128
NCORES = 8
VOCAB = 3100
ID_OFFSET = 1500
NUM_CLASSES = 100
BN_EPS = 1e-5

CFG = dict(
    n_nodes=100_000,
    n_graphs=512,
    win_rows=32_768,   # int16 index reach per gather window
    blk=512,           # dst nodes per PSUM block (one f32 bank)
    call_tiles=8,      # max 128-row tiles per dma_gather call (HW cap: 1024 idxs)
    oh_batch=4,        # one-hot tiles built per DVE op
)

_PROFILE = False
_LAST_EXEC_NS = None


def _install_profile_hook():
    try:
        import antenv
        from trn_agent_boot.trn_boot import _ntff_profile_via_ctypes
    except Exception:
        return False
    if "antenv.axon_hooks" in sys.modules:
        return True
    hooks = types.ModuleType("antenv.axon_hooks")
    hooks._hook = _ntff_profile_via_ctypes("/opt/axon/libaxon_pjrt.so")
    hooks.set_axon_ntff_profile_hook = lambda h: setattr(hooks, "_hook", h)
    hooks.get_axon_ntff_profile_hook = lambda: hooks._hook
    sys.modules["antenv.axon_hooks"] = hooks
    antenv.axon_hooks = hooks
    return True


def _derived(cfg):
    n, blk = cfg["n_nodes"], cfg["blk"]
    loc = n // NCORES
    lpad = ((loc + blk - 1) // blk) * blk
    nblk = lpad // blk
    ntn = lpad // P
    vpad = ((VOCAB - ID_OFFSET + P - 1) // P) * P   # 1664
    nvc = vpad // P
    nwin = (n + cfg["win_rows"] - 1) // cfg["win_rows"]
    nbin = 1
    return loc, lpad, nblk, ntn, vpad, nvc, nwin, nbin


class Sched:
    """Compile-time (core-independent) gather/scatter schedule."""
    pass


def _schedule(cfg, edge_src, edge_dst):
    """Build shared schedule from per-(block,window,bin) caps across cores."""
    loc, lpad, nblk, ntn, vpad, nvc, nwin, nbin = _derived(cfg)
    n, wr, blk = cfg["n_nodes"], cfg["win_rows"], cfg["blk"]
    MAXT = cfg["call_tiles"]
    ngrp = nblk * nwin * nbin

    cnts = np.zeros((NCORES, ngrp), np.int64)
    for c in range(NCORES):
        base = c * loc
        m = (edge_dst >= base) & (edge_dst < base + loc)
        src = edge_src[m]
        dl = edge_dst[m] - base
        b = dl // blk
        w = src // wr
        key = b * nwin + w
        cnts[c] = np.bincount(key, minlength=ngrp)

    cap = ((cnts.max(axis=0) + P - 1) // P) * P  # [ngrp]
    cap = cap.reshape(nblk, nwin, nbin)
    for b in range(nblk):
        if cap[b].sum() == 0:
            cap[b, 0, 0] = P  # pad tile so the block's PSUM bank gets started

    # tile order: (block, window, bin); calls chop at window changes / MAXT
    s = Sched()
    s.grp_slot0 = {}          # (b, w, bn) -> slot0
    s.tiles = []              # (block, col0, width, start, stop, win)
    slot0 = 0
    first_t, last_t = {}, {}
    for b in range(nblk):
        for w in range(nwin):
            for bn in range(nbin):
                cp = int(cap[b, w, bn])
                if cp == 0:
                    continue
                s.grp_slot0[(b, w, bn)] = slot0
                for _t in range(cp // P):
                    ti = len(s.tiles)
                    if b not in first_t:
                        first_t[b] = ti
                    last_t[b] = ti
                    s.tiles.append([b, 0, blk, False, False, w])
                    slot0 += P
    for b, ti in first_t.items():
        s.tiles[ti][3] = True
    for b, ti in last_t.items():
        s.tiles[ti][4] = True

    s.calls = []              # dict(win, tile0, ntiles)
    t0 = 0
    for t in range(len(s.tiles) + 1):
        flush = (t == len(s.tiles) or t - t0 == MAXT
                 or (t > t0 and s.tiles[t][5] != s.tiles[t0][5]))
        if flush and t > t0:
            s.calls.append(dict(win=s.tiles[t0][5], tile0=t0, ntiles=t - t0))
            t0 = t
    s.nslots = slot0
    s.ntiles = len(s.tiles)
    s.cap = cap
    s.max_call_tiles = max(c["ntiles"] for c in s.calls)
    return s


def _prep(cfg, s, node_ids, edge_src, edge_dst, graph_ids, Ws, bs, bn_gamma,
          bn_beta, eps, W_out, b_out, emb):
    """Index-only host preprocessing -> per-core input dicts."""
    loc, lpad, nblk, ntn, vpad, nvc, nwin, nbin = _derived(cfg)
    n, wr, blk = cfg["n_nodes"], cfg["win_rows"], cfg["blk"]
    node_ids = np.asarray(node_ids, np.int64)
    edge_src = np.asarray(edge_src, np.int64)
    edge_dst = np.asarray(edge_dst, np.int64)
    graph_ids = np.asarray(graph_ids, np.int64)

    tile_block = np.array([t[0] for t in s.tiles], np.int64)
    tile_col0 = np.array([t[1] for t in s.tiles], np.int64)
    tile_wide = np.array([t[2] == blk for t in s.tiles], bool)

    per_core = []
    for c in range(NCORES):
        base = c * loc
        m = (edge_dst >= base) & (edge_dst < base + loc)
        src = edge_src[m]
        dl = edge_dst[m] - base
        b = dl // blk
        w = src // wr

        # slot assignment: edges of group g sorted by dl, packed from grp_slot0
        gkey = b * nwin + w
        order = np.lexsort((dl, gkey))
        src, dl, w, gkey = (a[order] for a in (src, dl, w, gkey))
        slots = np.empty(len(src), np.int64)
        gs, gidx = np.unique(gkey, return_index=True)
        bounds = list(gidx) + [len(src)]
        for gi, g in enumerate(gs):
            lo, hi = bounds[gi], bounds[gi + 1]
            bb = int(g) // (nwin * nbin)
            ww = (int(g) // nbin) % nwin
            bnn = int(g) % nbin
            s0 = s.grp_slot0[(bb, ww, bnn)]
            slots[lo:hi] = s0 + np.arange(hi - lo)

        idx_flat = np.zeros(s.nslots, np.int16)
        idx_flat[slots] = (src - w * wr).astype(np.int16)
        edst_flat = np.full(s.nslots, -1.0, np.float32)
        tl = slots // P
        edst_flat[slots] = (dl - tile_block[tl] * blk).astype(np.float32)

        sl = np.arange(s.nslots)
        # gather idx layout: slot i -> (partition i%16, col i//16), replicated
        # across the 8 Q7 cores (16-partition groups)
        idx_w = np.zeros((16, s.nslots // 16), np.int16)
        idx_w[sl % 16, sl // 16] = idx_flat
        idx_w = np.tile(idx_w, (8, 1))
        edst_t = np.zeros((P, s.ntiles), np.float32)
        edst_t[sl % P, sl // P] = edst_flat

        # vocab count matrix cntT [vpad, lpad] fp16
        v = node_ids[src]
        cnt = np.bincount(v * lpad + dl, minlength=vpad * lpad)
        cntT = cnt.reshape(vpad, lpad).astype(np.float16)

        nidrep = np.full((P, lpad), -1.0, np.float16)
        nidrep[:, :loc] = np.tile(node_ids[base:base + loc].astype(np.float16),
                                  (P, 1))
        gl = np.full((P, ntn), -1.0, np.float32)
        g_base = int(graph_ids[base])
        g_span = int(graph_ids[base + loc - 1]) - g_base
        assert g_span < P, f"graph window {g_span} >= {P}"
        glv = (graph_ids[base:base + loc] - g_base).astype(np.float32)
        gl_full = np.full(lpad, -1.0, np.float32)
        gl_full[:loc] = glv
        gl[:, :] = gl_full.reshape(ntn, P).T
        growidx = np.minimum(g_base + np.arange(P), cfg["n_graphs"]).astype(
            np.int32)[:, None]

        wpack = np.zeros((HID, 9 * HID), np.float32)
        for l in range(3):
            for mm in range(3):
                wpack[:, (3 * l + mm) * HID:(3 * l + mm + 1) * HID] = Ws[l, mm]
        woutp = np.zeros((HID, 3 * NUM_CLASSES), np.float32)
        for l in range(3):
            woutp[:, l * NUM_CLASSES:(l + 1) * NUM_CLASSES] = \
                W_out[l * HID:(l + 1) * HID]

        per_core.append(dict(
            emb=np.asarray(emb, np.float32),
            cntT=cntT,
            nidrep=nidrep,
            idxw=idx_w,
            edstt=edst_t,
            gl=gl,
            growidx=growidx,
            iotaV=(np.arange(P, dtype=np.float32)[:, None]
                   + P * np.arange(nvc, dtype=np.float32)[None, :]),
            iota512=np.tile(np.arange(blk, dtype=np.float32)[None, :], (P, 1)),
            iota128=np.tile(np.arange(P, dtype=np.float32)[None, :], (P, 1)),
            idn64=np.eye(HID, dtype=np.float16),
            wpack=wpack,
            bsT=np.asarray(bs, np.float32).reshape(9, HID).T.copy(),
            gammaT=np.asarray(bn_gamma, np.float32).T.copy(),
            betaT=np.asarray(bn_beta, np.float32).T.copy(),
            eps_rep=np.tile(np.asarray(eps, np.float32)[None, :], (P, 1)),
            woutp=woutp,
            boutr=np.tile(np.asarray(b_out, np.float32)[None, :], (P, 1)),
        ))
    return per_core


def _build(cfg, s):
    import concourse.bacc as bacc
    import concourse.bass as bass
    import concourse.mybir as mybir
    import concourse.tile as tile

    loc, lpad, nblk, ntn, vpad, nvc, nwin, nbin = _derived(cfg)
    n, g, wr, blk = (cfg["n_nodes"], cfg["n_graphs"], cfg["win_rows"],
                     cfg["blk"])
    OHB = cfg["oh_batch"]
    f32, f16, i32 = mybir.dt.float32, mybir.dt.float16, mybir.dt.int32
    i16 = mybir.dt.int16
    AL, AF = mybir.AluOpType, mybir.ActivationFunctionType

    nc = bacc.Bacc()
    D = {}
    def di(name, shape, dt):
        D[name] = nc.dram_tensor(name, shape, dt, kind="ExternalInput")
        return D[name]

    emb = di("emb", [VOCAB, HID], f32)
    cntT = di("cntT", [vpad, lpad], f16)
    nidrep = di("nidrep", [P, lpad], f16)
    idxw = di("idxw", [P, s.nslots // 16], i16)
    edstt = di("edstt", [P, s.ntiles], f32)
    gl = di("gl", [P, ntn], f32)
    growidx = di("growidx", [P, 1], i32)
    iotaV = di("iotaV", [P, nvc], f32)
    iota512 = di("iota512", [P, blk], f32)
    iota128 = di("iota128", [P, P], f32)
    idn64 = di("idn64", [HID, HID], f16)
    wpack = di("wpack", [HID, 9 * HID], f32)
    bsT = di("bsT", [HID, 9], f32)
    gammaT = di("gammaT", [HID, 3], f32)
    betaT = di("betaT", [HID, 3], f32)
    eps_rep = di("eps_rep", [P, 3], f32)
    woutp = di("woutp", [HID, 3 * NUM_CLASSES], f32)
    boutr = di("boutr", [P, NUM_CLASSES], f32)

    hdev = [nc.dram_tensor(f"hdev{i}", [loc, P], f16) for i in range(2)]
    hfull = [nc.dram_tensor(f"hfull{i}", [n, P], f16, addr_space="Shared")
             for i in range(2)]
    statsin = [nc.dram_tensor(f"statsin{i}", [HID, 2], f32) for i in range(3)]
    statsout = [nc.dram_tensor(f"statsout{i}", [HID, 2], f32,
                               addr_space="Shared") for i in range(3)]
    obig = nc.dram_tensor("obig", [g + 1, NUM_CLASSES], f32)
    obig_red = nc.dram_tensor("obig_red", [g + 1, NUM_CLASSES], f32,
                              addr_space="Shared")
    out = nc.dram_tensor("out", [g, NUM_CLASSES], f32, kind="ExternalOutput")

    RG = [list(range(NCORES))]

    with tile.TileContext(nc) as tc:
        with (
            tc.tile_pool(name="c1", bufs=1) as c1,
            tc.tile_pool(name="cnt", bufs=3) as cntp,
            tc.tile_pool(name="rhs", bufs=3) as rhsp,
            tc.tile_pool(name="gat", bufs=3) as gatp,
            tc.tile_pool(name="idxp", bufs=3) as idxp,
            tc.tile_pool(name="oh", bufs=3) as ohp,
            tc.tile_pool(name="sb", bufs=4) as sbp,
            tc.tile_pool(name="rows", bufs=3) as rowsp,
            tc.tile_pool(name="psx", bufs=3, space="PSUM") as psx,
            tc.tile_pool(name="psm", bufs=2, space="PSUM") as psm,
            tc.tile_pool(name="pst", bufs=2, space="PSUM") as pst,
            tc.tile_pool(name="psp", bufs=1, space="PSUM") as psp,
        ):
            # ---- constants to SBUF ----
            def load(tname, dram, shape, dt):
                t = c1.tile(shape, dt, tag=tname)
                nc.sync.dma_start(out=t[:], in_=dram[:])
                return t
            nid_sb = load("nidrep", nidrep, [P, lpad], f16)
            edst_sb = load("edstt", edstt, [P, s.ntiles], f32)
            gl_sb = load("gl", gl, [P, ntn], f32)
            grow_sb = load("growidx", growidx, [P, 1], i32)
            iV_sb = load("iotaV", iotaV, [P, nvc], f32)
            i512_sb = load("iota512", iota512, [P, blk], f32)
            i128_sb = load("iota128", iota128, [P, P], f32)
            idn_sb = load("idn64", idn64, [HID, HID], f16)
            w_sb = load("wpack", wpack, [HID, 9 * HID], f32)
            b_sb = load("bsT", bsT, [HID, 9], f32)
            gam_sb = load("gammaT", gammaT, [HID, 3], f32)
            bet_sb = load("betaT", betaT, [HID, 3], f32)
            eps_sb = load("eps_rep", eps_rep, [P, 3], f32)
            wo_sb = load("woutp", woutp, [HID, 3 * NUM_CLASSES], f32)
            bo_sb = load("boutr", boutr, [P, NUM_CLASSES], f32)

            e1p32 = c1.tile([P, 3], f32)
            nc.vector.tensor_scalar(out=e1p32[:], in0=eps_sb[:], scalar1=1.0,
                                    scalar2=None, op0=AL.add)

            # ---- emb -> fp16 vocab chunks in SBUF [P, nvc*HID] ----
            embc = c1.tile([P, nvc * HID], f16)
            zt16 = c1.tile([P, HID], f16)
            nc.vector.memset(zt16[:], 0.0)
            for vc in range(nvc):
                r0 = ID_OFFSET + vc * P
                r1 = min(r0 + P, VOCAB)
                if r1 > r0:
                    et = rowsp.tile([P, HID], f32, tag="embcast")
                    nc.sync.dma_start(out=et[:r1 - r0, :], in_=emb[r0:r1, :])
                    nc.vector.tensor_copy(out=embc[:r1 - r0,
                                                   vc * HID:(vc + 1) * HID],
                                          in_=et[:r1 - r0, :])
                    if r1 - r0 < P:
                        nc.vector.tensor_copy(
                            out=embc[r1 - r0:, vc * HID:(vc + 1) * HID],
                            in_=zt16[:P - (r1 - r0), :])
                else:
                    nc.vector.tensor_copy(out=embc[:, vc * HID:(vc + 1) * HID],
                                          in_=zt16[:])

            hTown = c1.tile([HID, lpad], f16)
            x3f = c1.tile([HID, lpad], f16)
            pooledT = c1.tile([HID, 3 * P], f32)

            def mlp_block(l, cur, bs0):
                """3x(Linear+ReLU) on feature-major block; writes x3f slice."""
                for m in range(3):
                    ps_m = psm.tile([HID, blk], f32, space="PSUM", tag="psm")
                    nc.tensor.matmul(
                        ps_m[:],
                        lhsT=w_sb[:, (3 * l + m) * HID:(3 * l + m + 1) * HID],
                        rhs=cur, start=True, stop=True)
                    if m == 2:
                        dst_t = x3f[:, bs0:bs0 + blk]
                    else:
                        nxt = sbp.tile([HID, blk], f32, tag="mlp")
                        dst_t = nxt[:]
                    nc.scalar.activation(
                        out=dst_t, in_=ps_m[:], func=AF.Relu,
                        bias=b_sb[:, 3 * l + m:3 * l + m + 1])
                    cur = dst_t if m == 2 else nxt[:]

            # ================= layers =================
            for l in range(3):
                if l == 0:
                    for b in range(nblk):
                        bs0 = b * blk
                        ps_x = psx.tile([HID, blk], f32, space="PSUM",
                                        tag="psx")
                        for vc in range(nvc):
                            ct = cntp.tile([P, blk], f16)
                            nc.sync.dma_start(
                                out=ct[:],
                                in_=cntT[vc * P:(vc + 1) * P, bs0:bs0 + blk])
                            r2 = rhsp.tile([P, blk], f16)
                            nc.vector.tensor_scalar(
                                out=r2[:], in0=nid_sb[:, bs0:bs0 + blk],
                                scalar1=iV_sb[:, vc:vc + 1],
                                scalar2=e1p32[:, 0:1],
                                op0=AL.is_equal, op1=AL.mult)
                            nc.vector.tensor_tensor(
                                out=r2[:], in0=r2[:], in1=ct[:], op=AL.add)
                            nc.tensor.matmul(
                                ps_x[:],
                                lhsT=embc[:, vc * HID:(vc + 1) * HID],
                                rhs=r2[:],
                                start=(vc == 0), stop=(vc == nvc - 1))
                        xT = sbp.tile([HID, blk], f32, tag="xT")
                        nc.vector.tensor_copy(out=xT[:], in_=ps_x[:])
                        mlp_block(l, xT[:], bs0)
                else:
                    hprev = hfull[l - 1]
                    live_ps = {}
                    for call in s.calls:
                        w = call["win"]
                        w0 = w * wr
                        w1 = min(w0 + wr, n)
                        t0c, nt_c = call["tile0"], call["ntiles"]
                        ns_c = nt_c * P
                        it = idxp.tile([P, s.max_call_tiles * P // 16], i16,
                                       tag="idx")
                        nc.sync.dma_start(
                            out=it[:, :ns_c // 16],
                            in_=idxw[:, t0c * P // 16:
                                     (t0c * P + ns_c) // 16])
                        gt = gatp.tile([P, s.max_call_tiles * P], f16,
                                       tag="gat")
                        g0 = gt[:]
                        gat3 = bass.AP(g0.tensor, g0.offset,
                                       [g0.ap[0], [P, nt_c], [1, P]])
                        nc.gpsimd.dma_gather(
                            out_ap=gat3,
                            in_ap=hprev[w0:w1, :],
                            idxs_ap=it[:, :ns_c // 16],
                            num_idxs=ns_c,
                            num_idxs_reg=ns_c,
                            elem_size=P)

                        ti = 0
                        while ti < nt_c:
                            nb = min(OHB, nt_c - ti)
                            oh = ohp.tile([P, OHB * blk], f16, tag="oh")
                            a0 = edst_sb[:, t0c + ti:t0c + ti + nb]
                            in0 = bass.AP(a0.tensor, a0.offset,
                                          [a0.ap[0], [1, nb], [0, blk]])
                            a1 = i512_sb[:]
                            in1 = bass.AP(a1.tensor, a1.offset,
                                          [a1.ap[0], [0, nb], [1, blk]])
                            nc.vector.tensor_tensor(
                                out=oh[:, :nb * blk], in0=in0, in1=in1,
                                op=AL.is_equal)
                            for k in range(nb):
                                tk = t0c + ti + k
                                bk, c0k, wk, stk, spk = s.tiles[tk][:5]
                                if stk:
                                    ps_a = psx.tile([HID, blk], f32,
                                                    space="PSUM", tag="psx")
                                    live_ps[bk] = ps_a
                                else:
                                    ps_a = live_ps[bk]
                                nc.tensor.matmul(
                                    ps_a[:],
                                    lhsT=gt[:, (ti + k) * P:
                                            (ti + k) * P + HID],
                                    rhs=oh[:, k * blk:(k + 1) * blk],
                                    start=stk, stop=spk)
                                if spk:
                                    ps_d = live_ps.pop(bk)
                                    bs0 = bk * blk
                                    xT = sbp.tile([HID, blk], f32, tag="xT")
                                    nc.vector.tensor_scalar(
                                        out=xT[:], in0=hTown[:, bs0:bs0 + blk],
                                        scalar1=e1p32[:HID, l:l + 1],
                                        scalar2=None, op0=AL.mult)
                                    nc.vector.tensor_tensor(
                                        out=xT[:], in0=xT[:], in1=ps_d[:],
                                        op=AL.add)
                                    mlp_block(l, xT[:], bs0)
                            ti += nb
                    assert not live_ps, f"unfinished blocks {list(live_ps)}"

                # ---- BN stats + AllReduce ----
                st_t = sbp.tile([HID, 2], f32, tag="st")
                nc.vector.tensor_reduce(out=st_t[:, 0:1], in_=x3f[:, :loc],
                                        axis=mybir.AxisListType.X, op=AL.add)
                nc.scalar.activation(out=hTown[:, :loc], in_=x3f[:, :loc],
                                     func=AF.Square, accum_out=st_t[:, 1:2])
                nc.sync.dma_start(out=statsin[l][:], in_=st_t[:])
                nc.gpsimd.collective_compute(
                    "AllReduce", AL.add, replica_groups=RG,
                    ins=[statsin[l][:]], outs=[statsout[l][:]])
                rd = sbp.tile([HID, 2], f32, tag="rd")
                nc.sync.dma_start(out=rd[:], in_=statsout[l][:])
                mv = sbp.tile([HID, 1], f32, tag="mv")
                nc.vector.tensor_scalar(out=mv[:], in0=rd[:, 0:1],
                                        scalar1=1.0 / n, scalar2=None,
                                        op0=AL.mult)
                vr = sbp.tile([HID, 1], f32, tag="vr")
                nc.vector.tensor_scalar(out=vr[:], in0=rd[:, 1:2],
                                        scalar1=1.0 / n, scalar2=None,
                                        op0=AL.mult)
                m2 = sbp.tile([HID, 1], f32, tag="m2")
                nc.vector.tensor_tensor(out=m2[:], in0=mv[:], in1=mv[:],
                                        op=AL.mult)
                nc.vector.tensor_tensor(out=vr[:], in0=vr[:], in1=m2[:],
                                        op=AL.subtract)
                nc.vector.tensor_scalar(out=vr[:], in0=vr[:], scalar1=BN_EPS,
                                        scalar2=None, op0=AL.add)
                sq = sbp.tile([HID, 1], f32, tag="sq")
                nc.scalar.activation(out=sq[:], in_=vr[:], func=AF.Sqrt)
                inv = sbp.tile([HID, 1], f32, tag="inv")
                nc.vector.reciprocal(out=inv[:], in_=sq[:])
                scl = sbp.tile([HID, 1], f32, tag="scl")
                nc.vector.tensor_tensor(out=scl[:], in0=inv[:],
                                        in1=gam_sb[:, l:l + 1], op=AL.mult)
                shf = sbp.tile([HID, 1], f32, tag="shf")
                nc.vector.tensor_tensor(out=shf[:], in0=mv[:], in1=scl[:],
                                        op=AL.mult)
                nc.vector.tensor_tensor(out=shf[:], in0=bet_sb[:, l:l + 1],
                                        in1=shf[:], op=AL.subtract)
                # h_{l+1} = x3*scl + shf  (overwrites hTown)
                nc.vector.tensor_scalar(out=hTown[:], in0=x3f[:],
                                        scalar1=scl[:], scalar2=shf[:],
                                        op0=AL.mult, op1=AL.add)

                # ---- rows pass: transpose, pool, writeback ----
                ps_p = psp.tile([HID, P], f32, space="PSUM", tag="psp")
                for nt in range(ntn):
                    ps_t = pst.tile([P, HID], f16, space="PSUM", tag="pst")
                    nc.tensor.transpose(ps_t[:], hTown[:, nt * P:(nt + 1) * P],
                                        idn_sb[:])
                    r16 = rowsp.tile([P, HID], f16, tag="r16")
                    nc.scalar.activation(out=r16[:], in_=ps_t[:], func=AF.Copy)
                    if l < 2:
                        nr = min(P, loc - nt * P)
                        if nr > 0:
                            nc.sync.dma_start(
                                out=hdev[l][nt * P:nt * P + nr, 0:HID],
                                in_=r16[:nr, :])
                    gh = rowsp.tile([P, P], f16, tag="gh")
                    nc.vector.tensor_scalar(out=gh[:], in0=i128_sb[:],
                                            scalar1=gl_sb[:, nt:nt + 1],
                                            scalar2=None, op0=AL.is_equal)
                    nc.tensor.matmul(ps_p[:], lhsT=r16[:], rhs=gh[:],
                                     start=(nt == 0), stop=(nt == ntn - 1))
                nc.vector.tensor_copy(out=pooledT[:, l * P:(l + 1) * P],
                                      in_=ps_p[:])
                if l < 2:
                    nc.gpsimd.collective_compute(
                        "AllGather", AL.bypass, replica_groups=RG,
                        ins=[hdev[l][:]], outs=[hfull[l][:]])

            # ================= final linear =================
            ps_o = psp.tile([P, NUM_CLASSES], f32, space="PSUM", tag="psp")
            for l in range(3):
                nc.tensor.matmul(
                    ps_o[:], lhsT=pooledT[:, l * P:(l + 1) * P],
                    rhs=wo_sb[:, l * NUM_CLASSES:(l + 1) * NUM_CLASSES],
                    start=(l == 0), stop=(l == 2))
            oloc = sbp.tile([P, NUM_CLASSES], f32, tag="oloc")
            nc.vector.tensor_copy(out=oloc[:], in_=ps_o[:])
            zt = sbp.tile([P, NUM_CLASSES], f32, tag="zt")
            nc.vector.memset(zt[:], 0.0)
            nzt = (g + 1 + P - 1) // P
            for r in range(nzt):
                r0 = r * P
                r1 = min(r0 + P, g + 1)
                nc.sync.dma_start(out=obig[r0:r1, :], in_=zt[:r1 - r0, :])
            nc.gpsimd.indirect_dma_start(
                out=obig[:], out_offset=bass.IndirectOffsetOnAxis(
                    ap=grow_sb[:, 0:1], axis=0),
                in_=oloc[:], in_offset=None)
            nc.gpsimd.collective_compute(
                "AllReduce", AL.add, replica_groups=RG,
                ins=[obig[:]], outs=[obig_red[:]])
            for r in range((g + P - 1) // P):
                r0 = r * P
                r1 = min(r0 + P, g)
                ot = sbp.tile([P, NUM_CLASSES], f32, tag="ot")
                nc.sync.dma_start(out=ot[:r1 - r0, :], in_=obig_red[r0:r1, :])
                nc.vector.tensor_tensor(out=ot[:r1 - r0, :], in0=ot[:r1 - r0, :],
                                        in1=bo_sb[:r1 - r0, :], op=AL.add)
                nc.sync.dma_start(out=out[r0:r1, :], in_=ot[:r1 - r0, :])
    return nc


def build_all(cfg, inputs):
    s = _schedule(cfg, np.asarray(inputs["edge_src"], np.int64),
                  np.asarray(inputs["edge_dst"], np.int64))
    per_core = _prep(
        cfg, s, inputs["node_ids"], inputs["edge_src"], inputs["edge_dst"],
        inputs["graph_ids"], inputs["Ws"], inputs["bs"], inputs["bn_gamma"],
        inputs["bn_beta"], inputs["eps"], inputs["W_out"], inputs["b_out"],
        inputs["emb"])
    nc = _build(cfg, s)
    return nc, per_core


def kernel(**inputs):
    global _LAST_EXEC_NS
    import concourse.bass_utils as bass_utils
    bass_utils.upload_artifacts = lambda tmpdir: tmpdir
    from concourse.bass_utils import run_bass_kernel_spmd

    nc, per_core = build_all(CFG, inputs)
    nc.finalize()
    trace = bool(_PROFILE) and _install_profile_hook()
    res = run_bass_kernel_spmd(nc, per_core, core_ids=list(range(NCORES)),
                               trace=trace)
    _LAST_EXEC_NS = res.exec_time_ns
    return np.asarray(res.results[0]["out"], np.float32)
